# revision 20
# baseline (speedup 1.0000x reference)
"""Trainium2 Bass kernel for the DataReloadingQNN problem.

Math: layers 0..4 and the shared layer-5 gates B_q = RZ RY RZ are
sample-independent -> shared state v.  The per-sample part is
    state_b = P . prod_q RY_q(x_bq) . v          (P = CNOT chain)
with RY_q = c_q I + s_q J_q.  Qubits E = {0, 3..10} are expanded into a
K=512 matmul  t_b = sum_m W[b,m] u_m  (u_m = J^m v, P folded into the
column order); qubits 1 and 2 are applied afterwards as per-sample
butterflies.

Column storage uses the GRAY CODE of the P-space index.  P is the
prefix-XOR map, so the butterfly pairing for qubit q is a XOR with the
low-ones mask 2^(11-q)-1, and the sign of the s-term is bit_q ^
bit_{q-1}.  Gray coding maps that pairing to a SINGLE bit flip (an
aligned block swap at stride 2^(10-q)) and the sign to exactly that
storage bit:
    out[bit=0 blocks] = c*y - s*y_swap      (contiguous, uniform sign)
    out[bit=1 blocks] = c*y + s*y_swap
Qubit 2 -> stride 256, qubit 1 -> stride 512: both land inside a
1024-column g-half, so each PSUM half evacuates and finishes
independently (short pipeline tail).  The host un-grays the columns at
the end (cheap gather).

Host precomputes U (params-derived, replicated) and W^T (per-sample
trig products, pre-transposed) so the device does only: matmuls (K=512
in 4 PSUM-accumulated k-tiles), the two butterflies, and the out DMA.
Engine split per (tile, half): ScalarE does the s2-scaled PSUM read
(+ the c1 scale on odd halves), VectorE does the q2 combine phase A
(scalar_tensor_tensor), the q1 scales and combines, GpSimd does the q2
combine phase B.  Inputs sharded batch-wise across 8 cores.
"""
import numpy as np
import ml_dtypes

import concourse.bass as bass
import concourse.bacc as bacc
import concourse.tile as tile
from concourse import mybir
from concourse.bass_utils import run_bass_kernel_spmd

N = 11
DIM = 2048
BATCH = 8192
NCORES = 8
BSH = BATCH // NCORES          # 1024 samples per core
NTILES = BSH // 128            # 8 sample-tiles per core
PULL = (1, 2)                  # butterfly qubits
E = tuple(q for q in range(N) if q not in PULL)   # expanded, bit j <-> E[j]
K = 512
KT = K // 128                  # 4 k-tiles
F32 = mybir.dt.float32
BF16 = mybir.dt.bfloat16

# ---------------------------------------------------------------- host math


def _rz(phi):
    e = np.exp(-0.5j * phi)
    return np.array([[e, 0], [0, np.conj(e)]], dtype=np.complex128)


def _ry(theta):
    t = 0.5 * theta
    c, s = np.cos(t), np.sin(t)
    return np.array([[c, -s], [s, c]], dtype=np.complex128)


def _apply_1q_rows(rows, U, q):
    R = rows.shape[0]
    st = rows.reshape(R, 2 ** q, 2, 2 ** (N - 1 - q))
    st = np.einsum('ab,rxby->rxay', U, st)
    return st.reshape(R, DIM)


def _apply_cnot_rows(rows, c):
    R = rows.shape[0]
    st = rows.reshape(R, 2 ** c, 2, 2, 2 ** (N - 2 - c))
    st = np.stack([st[:, :, 0], st[:, :, 1, ::-1]], axis=2)
    return st.reshape(R, DIM)


def _tables():
    idx = np.arange(DIM)
    gray = idx ^ (idx >> 1)
    a_of_g = np.zeros(DIM, dtype=np.int64)
    a_of_g[gray] = idx
    g = np.arange(DIM)[None, :].astype(np.float64)
    for c in range(N - 1):
        g = _apply_cnot_rows(g, c)
    pimg = g[0].astype(np.int64)
    return gray, pimg[a_of_g]


GRAY, COLMAP = _tables()


def build_u(params):
    """(6,11,3) f32 -> u_dev (2, 4, 128, 2048) bf16.

    u_dev[h, k, p, :] = row m=128k+p of U in storage order, columns
    [re g(1024h:+1024) | im g(1024h:+1024)].
    """
    p = params.astype(np.float64)
    v = np.zeros((1, DIM), dtype=np.complex128)
    v[0, 0] = 1.0
    for l in range(5):
        for q in range(N):
            v = _apply_1q_rows(v, _rz(p[l, q, 0]), q)
            v = _apply_1q_rows(v, _ry(p[l, q, 1]), q)
            v = _apply_1q_rows(v, _rz(p[l, q, 2]), q)
        for c in range(N - 1):
            v = _apply_cnot_rows(v, c)
    for q in range(N):
        B = _rz(p[5, q, 2]) @ _ry(p[5, q, 1]) @ _rz(p[5, q, 0])
        v = _apply_1q_rows(v, B, q)
    J = np.array([[0, -1], [1, 0]], dtype=np.complex128)
    rows = v
    for q in E:
        rc = _apply_1q_rows(rows, J, q)
        rows = np.concatenate([rows, rc], axis=0)
    rows = rows[:, COLMAP]
    # device column order per half: (b8, b9, pl, r) so every butterfly
    # phase is a contiguous slice; g = 1024*h + 512*b9 + 256*b8 + r
    u_dev = np.empty((2, KT, 128, DIM), dtype=np.float64)
    for h in range(2):
        re = rows.real[:, 1024 * h:1024 * h + 1024].reshape(K, 2, 2, 256)
        im = rows.imag[:, 1024 * h:1024 * h + 1024].reshape(K, 2, 2, 256)
        arr = np.stack([re, im], axis=2)          # (K, b9, pl, b8, r)
        arr = arr.transpose(0, 3, 1, 2, 4)        # (K, b8, b9, pl, r)
        u_dev[h] = arr.reshape(K, DIM).reshape(KT, 128, DIM)
    return np.ascontiguousarray(u_dev.astype(ml_dtypes.bfloat16))


def build_w(X):
    """(B, 11) f32 -> W (B, 512) f32; bit j of m <-> qubit E[j] (LSB=E[0])."""
    xh = X.astype(np.float64) * 0.5
    W = np.ones((X.shape[0], 1))
    for q in E:
        c, s = np.cos(xh[:, q])[:, None], np.sin(xh[:, q])[:, None]
        W = np.concatenate([W * c, W * s], axis=1)
    return W

# ------------------------------------------------------------- bass kernel


def build_kernel():
    nc = bacc.Bacc()
    trig_d = nc.dram_tensor("trig", (128, NTILES * 4), F32,
                            kind="ExternalInput")
    wt_d = nc.dram_tensor("wt", (KT, 128, BSH), BF16, kind="ExternalInput")
    u_d = nc.dram_tensor("u", (2, KT, 128, DIM), BF16, kind="ExternalInput")
    out_d = nc.dram_tensor("out", (BSH, 2 * DIM), BF16,
                           kind="ExternalOutput")

    MULT = mybir.AluOpType.mult
    ADD = mybir.AluOpType.add
    SUB = mybir.AluOpType.subtract

    with tile.TileContext(nc) as tc:
        with (
            tc.tile_pool(name="const", bufs=1) as const_pool,
            tc.tile_pool(name="uin", bufs=1) as u_pool,
            tc.tile_pool(name="ys", bufs=2) as y_pool,
            tc.tile_pool(name="tmps", bufs=2) as tmp_pool,
            tc.tile_pool(name="outs", bufs=4) as out_pool,
            tc.tile_pool(name="pmm", bufs=2,
                         space=bass.MemorySpace.PSUM) as pmm_pool,
        ):
            # absorb the activation-table load before the first real op
            junk = const_pool.tile([128, 1], F32)
            nc.scalar.mul(junk[:], junk[:], 1.0)

            trig = const_pool.tile([128, NTILES * 4], F32)
            junk_w = const_pool.tile([128, 640], BF16)
            wts = const_pool.tile([128, KT * 128 * NTILES], BF16)
            uts = [[u_pool.tile([128, DIM], BF16, tag=f"u{h}{k}",
                                name=f"u{h}{k}")
                    for k in range(KT)] for h in range(2)]

            # Load plan (per-queue streams ~130 GB/s): sync carries trig +
            # wt (critical for the first matmuls) and most stores; the
            # scalar/gpsimd queues split the U tiles by k-parity so the
            # first two k-tiles of h0 land as early as possible.
            nc.sync.dma_start(wts[:, 0:1024], wt_d[0])
            nc.scalar.dma_start(uts[0][0][:], u_d[0, 0])
            nc.gpsimd.dma_start(uts[0][1][:], u_d[0, 1])
            nc.sync.dma_start(trig[:], trig_d[:])
            nc.scalar.dma_start(uts[0][2][:], u_d[0, 2])
            nc.gpsimd.dma_start(uts[0][3][:], u_d[0, 3])
            nc.sync.dma_start(wts[:, 1024:2048], wt_d[1])
            nc.scalar.dma_start(uts[1][0][:], u_d[1, 0])
            nc.gpsimd.dma_start(uts[1][1][:], u_d[1, 1])
            nc.sync.dma_start(wts[:, 2048:3072], wt_d[2])
            nc.scalar.dma_start(uts[1][2][:], u_d[1, 2])
            nc.gpsimd.dma_start(uts[1][3][:], u_d[1, 3])
            nc.sync.dma_start(wts[:, 3072:4096], wt_d[3])

            # store queues: sync 10, gpsimd 6 (Pool sequencer is idle;
            # ScalarE is busy so its queue only carries startup loads)
            qs = [nc.sync] * 16
            for i in (2, 5, 8, 10, 13, 15):
                qs[i] = nc.gpsimd
            # warm the PE p-state with dummy matmuls on garbage SBUF while
            # the first loads stream in; start=True overwrites PSUM so the
            # junk never contaminates real accumulation
            nc.vector.memset(junk_w[:], 0.0)
            warm = pmm_pool.tile([128, DIM], F32, tag="pm", name="warm")
            for w in range(10):
                nc.tensor.matmul(warm[:, 0:512], junk_w[:, 0:128],
                                 junk_w[:, 128:640], start=True, stop=True)

            hidx = 0
            # h-major order: all h0 halves first so the h1 U tiles can
            # stream in behind the h0 compute
            for h in range(2):
                for t in range(NTILES):
                    c1 = trig[:, 4 * t + 0:4 * t + 1]
                    s1 = trig[:, 4 * t + 1:4 * t + 2]
                    c2 = trig[:, 4 * t + 2:4 * t + 3]
                    s2 = trig[:, 4 * t + 3:4 * t + 4]
                    pm = pmm_pool.tile([128, DIM], F32, tag="pm")
                    for k in range(KT):
                        lhsT = wts[:, 1024 * k + 128 * t:
                                   1024 * k + 128 * (t + 1)]
                        for c in range(4):
                            nc.tensor.matmul(
                                pm[:, 512 * c:512 * (c + 1)], lhsT,
                                uts[h][k][:, 512 * c:512 * (c + 1)],
                                start=(k == 0), stop=(k == KT - 1))
                    # columns of pm: 1024*b8 + 512*b9 + 256*pl + r.
                    # q2 butterfly (pairs b8) = contiguous 1024-halves.
                    # ScalarE makes both scaled PSUM copies (it reads PSUM
                    # at a flat rate and frees pm fast); DVE combines with
                    # contiguous 2x tensor_tensor ops.
                    ts = tmp_pool.tile([128, DIM], BF16, tag="ts")
                    tc = tmp_pool.tile([128, DIM], BF16, tag="tc")
                    nc.scalar.mul(ts[:], pm[:], s2)
                    nc.scalar.mul(tc[:], pm[:], c2)
                    y = y_pool.tile([128, DIM], BF16, tag="y")
                    nc.vector.tensor_tensor(
                        y[:, 0:1024], tc[:, 0:1024], ts[:, 1024:2048], SUB)
                    nc.gpsimd.tensor_tensor(
                        y[:, 1024:2048], tc[:, 1024:2048], ts[:, 0:1024],
                        ADD)
                    # q1 butterfly (pairs b9): scaled copies at 4x, then
                    # contiguous 512-wide combines, all on DVE
                    us = tmp_pool.tile([128, DIM], BF16, tag="us")
                    uc = tmp_pool.tile([128, DIM], BF16, tag="uc")
                    nc.vector.tensor_scalar_mul(us[:], y[:], s1)
                    nc.vector.tensor_scalar_mul(uc[:], y[:], c1)
                    oh = out_pool.tile([128, DIM], BF16, tag="oh")
                    for b8 in range(2):
                        o = 1024 * b8
                        nc.vector.tensor_tensor(
                            oh[:, o:o + 512], uc[:, o:o + 512],
                            us[:, o + 512:o + 1024], SUB)
                        nc.vector.tensor_tensor(
                            oh[:, o + 512:o + 1024], uc[:, o + 512:o + 1024],
                            us[:, o:o + 512], ADD)
                    # out columns 2048*h + pm-order; host unscrambles
                    qs[hidx].dma_start(
                        out_d[128 * t:128 * (t + 1),
                              DIM * h:DIM * (h + 1)], oh[:])
                    hidx += 1
    nc.finalize()
    return nc

# ----------------------------------------------------------------- driver

_CACHE = {}


def make_inputs(X, params):
    X = np.asarray(X, dtype=np.float32)
    params = np.asarray(params, dtype=np.float32)
    u_dev = build_u(params)
    W = build_w(X)
    xh = X.astype(np.float64) * 0.5
    trig_all = np.stack([np.cos(xh[:, 1]), np.sin(xh[:, 1]),
                         np.cos(xh[:, 2]), np.sin(xh[:, 2])],
                        axis=-1).astype(np.float32)   # (B, 4)
    in_maps = []
    for c in range(NCORES):
        sl = slice(c * BSH, (c + 1) * BSH)
        trig_np = np.ascontiguousarray(
            trig_all[sl].reshape(NTILES, 128, 4).transpose(1, 0, 2)
            .reshape(128, NTILES * 4))
        wt_np = np.ascontiguousarray(
            W[sl].T.reshape(KT, 128, BSH).astype(ml_dtypes.bfloat16))
        in_maps.append({"trig": trig_np, "wt": wt_np, "u": u_dev})
    return in_maps


def _out_index():
    # device col for (plane pl, g): h=g>>10, b9=(g>>9)&1, b8=(g>>8)&1, r=g&255
    # col = 2048*h + 1024*b8 + 512*b9 + 256*pl + r;  final[b,a,pl] uses g=GRAY[a]
    g = GRAY
    col = (2048 * (g >> 10) + 1024 * ((g >> 8) & 1) + 512 * ((g >> 9) & 1)
           + (g & 255))
    return np.stack([col, col + 256], axis=-1)     # (2048, 2)


OUT_IDX = _out_index()


def postprocess(results):
    flat = np.concatenate([results[c]["out"] for c in range(NCORES)],
                          axis=0).astype(np.float32)
    return flat[:, OUT_IDX]


def kernel(X, params):
    if "nc" not in _CACHE:
        _CACHE["nc"] = build_kernel()
    nc = _CACHE["nc"]
    in_maps = make_inputs(X, params)
    res = run_bass_kernel_spmd(nc, in_maps, list(range(NCORES)))
    return postprocess(res.results)


# revision 21
# speedup vs baseline: 1.4176x; 1.4176x over previous
"""Trainium2 Bass kernel for the DataReloadingQNN problem.

Math: layers 0..4 and the shared layer-5 gates B_q = RZ RY RZ are
sample-independent -> shared state v.  The per-sample part is
    state_b = P . prod_q RY_q(x_bq) . v          (P = CNOT chain)
with RY_q = c_q I + s_q J_q.  Qubits E = {0, 3..10} are expanded into a
K=512 matmul  t_b = sum_m W[b,m] u_m  (u_m = J^m v, P folded into the
column order); qubits 1 and 2 are applied afterwards as per-sample
butterflies.

Column storage uses the GRAY CODE of the P-space index.  P is the
prefix-XOR map, so the butterfly pairing for qubit q is a XOR with the
low-ones mask 2^(11-q)-1, and the sign of the s-term is bit_q ^
bit_{q-1}.  Gray coding maps that pairing to a SINGLE bit flip (an
aligned block swap at stride 2^(10-q)) and the sign to exactly that
storage bit:
    out[bit=0 blocks] = c*y - s*y_swap      (contiguous, uniform sign)
    out[bit=1 blocks] = c*y + s*y_swap
Qubit 2 -> stride 256, qubit 1 -> stride 512: both land inside a
1024-column g-half, so each PSUM half evacuates and finishes
independently (short pipeline tail).  The host un-grays the columns at
the end (cheap gather).

Host precomputes U (params-derived, replicated) and W^T (per-sample
trig products, pre-transposed) so the device does only: matmuls (K=512
in 4 PSUM-accumulated k-tiles), the two butterflies, and the out DMA.
Engine split per (tile, half): ScalarE does the s2-scaled PSUM read
(+ the c1 scale on odd halves), VectorE does the q2 combine phase A
(scalar_tensor_tensor), the q1 scales and combines, GpSimd does the q2
combine phase B.  Inputs sharded batch-wise across 8 cores.
"""
import numpy as np
import ml_dtypes

import concourse.bass as bass
import concourse.bacc as bacc
import concourse.tile as tile
from concourse import mybir
from concourse.bass_utils import run_bass_kernel_spmd

N = 11
DIM = 2048
BATCH = 8192
NCORES = 8
BSH = BATCH // NCORES          # 1024 samples per core
NTILES = BSH // 128            # 8 sample-tiles per core
PULL = (1, 2)                  # butterfly qubits
E = tuple(q for q in range(N) if q not in PULL)   # expanded, bit j <-> E[j]
K = 512
KT = K // 128                  # 4 k-tiles
F32 = mybir.dt.float32
BF16 = mybir.dt.bfloat16

# ---------------------------------------------------------------- host math


def _rz(phi):
    e = np.exp(-0.5j * phi)
    return np.array([[e, 0], [0, np.conj(e)]], dtype=np.complex128)


def _ry(theta):
    t = 0.5 * theta
    c, s = np.cos(t), np.sin(t)
    return np.array([[c, -s], [s, c]], dtype=np.complex128)


def _apply_1q_rows(rows, U, q):
    R = rows.shape[0]
    st = rows.reshape(R, 2 ** q, 2, 2 ** (N - 1 - q))
    st = np.einsum('ab,rxby->rxay', U, st)
    return st.reshape(R, DIM)


def _apply_cnot_rows(rows, c):
    R = rows.shape[0]
    st = rows.reshape(R, 2 ** c, 2, 2, 2 ** (N - 2 - c))
    st = np.stack([st[:, :, 0], st[:, :, 1, ::-1]], axis=2)
    return st.reshape(R, DIM)


def _tables():
    idx = np.arange(DIM)
    gray = idx ^ (idx >> 1)
    a_of_g = np.zeros(DIM, dtype=np.int64)
    a_of_g[gray] = idx
    g = np.arange(DIM)[None, :].astype(np.float64)
    for c in range(N - 1):
        g = _apply_cnot_rows(g, c)
    pimg = g[0].astype(np.int64)
    return gray, pimg[a_of_g]


GRAY, COLMAP = _tables()


def build_u(params):
    """(6,11,3) f32 -> u_dev (2, 4, 128, 2048) bf16.

    u_dev[h, k, p, :] = row m=128k+p of U in storage order, columns
    [re g(1024h:+1024) | im g(1024h:+1024)].
    """
    p = params.astype(np.float64)
    v = np.zeros((1, DIM), dtype=np.complex128)
    v[0, 0] = 1.0
    for l in range(5):
        for q in range(N):
            v = _apply_1q_rows(v, _rz(p[l, q, 0]), q)
            v = _apply_1q_rows(v, _ry(p[l, q, 1]), q)
            v = _apply_1q_rows(v, _rz(p[l, q, 2]), q)
        for c in range(N - 1):
            v = _apply_cnot_rows(v, c)
    for q in range(N):
        B = _rz(p[5, q, 2]) @ _ry(p[5, q, 1]) @ _rz(p[5, q, 0])
        v = _apply_1q_rows(v, B, q)
    J = np.array([[0, -1], [1, 0]], dtype=np.complex128)
    rows = v
    for q in E:
        rc = _apply_1q_rows(rows, J, q)
        rows = np.concatenate([rows, rc], axis=0)
    rows = rows[:, COLMAP]
    # device column order per half: (b8, b9, pl, r) so every butterfly
    # phase is a contiguous slice; g = 1024*h + 512*b9 + 256*b8 + r
    u_dev = np.empty((2, KT, 128, DIM), dtype=np.float64)
    for h in range(2):
        re = rows.real[:, 1024 * h:1024 * h + 1024].reshape(K, 2, 2, 256)
        im = rows.imag[:, 1024 * h:1024 * h + 1024].reshape(K, 2, 2, 256)
        arr = np.stack([re, im], axis=2)          # (K, b9, pl, b8, r)
        arr = arr.transpose(0, 3, 1, 2, 4)        # (K, b8, b9, pl, r)
        u_dev[h] = arr.reshape(K, DIM).reshape(KT, 128, DIM)
    return np.ascontiguousarray(u_dev.astype(ml_dtypes.bfloat16))


def build_w(X):
    """(B, 11) f32 -> W (B, 512) f32; bit j of m <-> qubit E[j] (LSB=E[0])."""
    xh = X.astype(np.float64) * 0.5
    W = np.ones((X.shape[0], 1))
    for q in E:
        c, s = np.cos(xh[:, q])[:, None], np.sin(xh[:, q])[:, None]
        W = np.concatenate([W * c, W * s], axis=1)
    return W

# ------------------------------------------------------------- bass kernel


def build_kernel():
    nc = bacc.Bacc()
    trig_d = nc.dram_tensor("trig", (128, NTILES * 4), F32,
                            kind="ExternalInput")
    wt_d = nc.dram_tensor("wt", (KT, 128, BSH), BF16, kind="ExternalInput")
    u_d = nc.dram_tensor("u", (2, KT, 128, DIM), BF16, kind="ExternalInput")
    out_d = nc.dram_tensor("out", (BSH, 2 * DIM), BF16,
                           kind="ExternalOutput")

    MULT = mybir.AluOpType.mult
    ADD = mybir.AluOpType.add
    SUB = mybir.AluOpType.subtract

    with tile.TileContext(nc) as tc:
        with (
            tc.tile_pool(name="const", bufs=1) as const_pool,
            tc.tile_pool(name="uin", bufs=1) as u_pool,
            tc.tile_pool(name="ys", bufs=2) as y_pool,
            tc.tile_pool(name="tmps", bufs=2) as tmp_pool,
            tc.tile_pool(name="outs", bufs=4) as out_pool,
            tc.tile_pool(name="pmm", bufs=2,
                         space=bass.MemorySpace.PSUM) as pmm_pool,
        ):
            # absorb the activation-table load before the first real op
            junk = const_pool.tile([128, 1], F32)
            nc.scalar.mul(junk[:], junk[:], 1.0)

            trig = const_pool.tile([128, NTILES * 4], F32)
            junk_w = const_pool.tile([128, 640], BF16)
            wts = const_pool.tile([128, KT * 128 * NTILES], BF16)
            uts = [[u_pool.tile([128, DIM], BF16, tag=f"u{h}{k}",
                                name=f"u{h}{k}")
                    for k in range(KT)] for h in range(2)]

            # Load plan (per-queue streams ~130 GB/s): sync carries trig +
            # wt (critical for the first matmuls) and most stores; the
            # scalar/gpsimd queues split the U tiles by k-parity so the
            # first two k-tiles of h0 land as early as possible.
            nc.sync.dma_start(wts[:, 0:1024], wt_d[0])
            nc.scalar.dma_start(uts[0][0][:], u_d[0, 0])
            nc.gpsimd.dma_start(uts[0][1][:], u_d[0, 1])
            nc.sync.dma_start(trig[:], trig_d[:])
            nc.scalar.dma_start(uts[0][2][:], u_d[0, 2])
            nc.gpsimd.dma_start(uts[0][3][:], u_d[0, 3])
            nc.sync.dma_start(wts[:, 1024:2048], wt_d[1])
            nc.scalar.dma_start(uts[1][0][:], u_d[1, 0])
            nc.gpsimd.dma_start(uts[1][1][:], u_d[1, 1])
            nc.sync.dma_start(wts[:, 2048:3072], wt_d[2])
            nc.scalar.dma_start(uts[1][2][:], u_d[1, 2])
            nc.gpsimd.dma_start(uts[1][3][:], u_d[1, 3])
            nc.sync.dma_start(wts[:, 3072:4096], wt_d[3])

            # store queues: sync 10, gpsimd 6 (Pool sequencer is idle;
            # ScalarE is busy so its queue only carries startup loads)
            qs = [nc.sync] * 16
            for i in (2, 5, 8, 10, 13, 15):
                qs[i] = nc.gpsimd
            # warm the PE p-state with dummy matmuls on garbage SBUF while
            # the first loads stream in; start=True overwrites PSUM so the
            # junk never contaminates real accumulation
            nc.vector.memset(junk_w[:], 0.0)
            warm = pmm_pool.tile([128, DIM], F32, tag="pm", name="warm")
            for w in range(10):
                nc.tensor.matmul(warm[:, 0:512], junk_w[:, 0:128],
                                 junk_w[:, 128:640], start=True, stop=True)

            hidx = 0
            # h-major order: all h0 halves first so the h1 U tiles can
            # stream in behind the h0 compute
            for h in range(2):
                for t in range(NTILES):
                    c1 = trig[:, 4 * t + 0:4 * t + 1]
                    s1 = trig[:, 4 * t + 1:4 * t + 2]
                    c2 = trig[:, 4 * t + 2:4 * t + 3]
                    s2 = trig[:, 4 * t + 3:4 * t + 4]
                    pm = pmm_pool.tile([128, DIM], F32, tag="pm")
                    for k in range(KT):
                        lhsT = wts[:, 1024 * k + 128 * t:
                                   1024 * k + 128 * (t + 1)]
                        for c in range(4):
                            nc.tensor.matmul(
                                pm[:, 512 * c:512 * (c + 1)], lhsT,
                                uts[h][k][:, 512 * c:512 * (c + 1)],
                                start=(k == 0), stop=(k == KT - 1))
                    # columns of pm: 1024*b8 + 512*b9 + 256*pl + r.
                    # q2 butterfly (pairs b8) = contiguous 1024-halves.
                    # ScalarE makes both scaled PSUM copies (it reads PSUM
                    # at a flat rate and frees pm fast); DVE combines with
                    # contiguous 2x tensor_tensor ops.
                    ts = tmp_pool.tile([128, DIM], BF16, tag="ts")
                    tc = tmp_pool.tile([128, DIM], BF16, tag="tc")
                    nc.scalar.mul(ts[:], pm[:], s2)
                    nc.scalar.mul(tc[:], pm[:], c2)
                    y = y_pool.tile([128, DIM], BF16, tag="y")
                    nc.vector.tensor_tensor(
                        y[:, 0:1024], tc[:, 0:1024], ts[:, 1024:2048], SUB)
                    nc.vector.tensor_tensor(
                        y[:, 1024:2048], tc[:, 1024:2048], ts[:, 0:1024],
                        ADD)
                    # q1 butterfly (pairs b9): scaled copies at 4x, then
                    # contiguous 512-wide combines, all on DVE
                    us = tmp_pool.tile([128, DIM], BF16, tag="us")
                    uc = tmp_pool.tile([128, DIM], BF16, tag="uc")
                    nc.vector.tensor_scalar_mul(us[:], y[:], s1)
                    nc.vector.tensor_scalar_mul(uc[:], y[:], c1)
                    oh = out_pool.tile([128, DIM], BF16, tag="oh")
                    for b8 in range(2):
                        o = 1024 * b8
                        nc.vector.tensor_tensor(
                            oh[:, o:o + 512], uc[:, o:o + 512],
                            us[:, o + 512:o + 1024], SUB)
                        nc.vector.tensor_tensor(
                            oh[:, o + 512:o + 1024], uc[:, o + 512:o + 1024],
                            us[:, o:o + 512], ADD)
                    # out columns 2048*h + pm-order; host unscrambles
                    qs[hidx].dma_start(
                        out_d[128 * t:128 * (t + 1),
                              DIM * h:DIM * (h + 1)], oh[:])
                    hidx += 1
    nc.finalize()
    return nc

# ----------------------------------------------------------------- driver

_CACHE = {}


def make_inputs(X, params):
    X = np.asarray(X, dtype=np.float32)
    params = np.asarray(params, dtype=np.float32)
    u_dev = build_u(params)
    W = build_w(X)
    xh = X.astype(np.float64) * 0.5
    trig_all = np.stack([np.cos(xh[:, 1]), np.sin(xh[:, 1]),
                         np.cos(xh[:, 2]), np.sin(xh[:, 2])],
                        axis=-1).astype(np.float32)   # (B, 4)
    in_maps = []
    for c in range(NCORES):
        sl = slice(c * BSH, (c + 1) * BSH)
        trig_np = np.ascontiguousarray(
            trig_all[sl].reshape(NTILES, 128, 4).transpose(1, 0, 2)
            .reshape(128, NTILES * 4))
        wt_np = np.ascontiguousarray(
            W[sl].T.reshape(KT, 128, BSH).astype(ml_dtypes.bfloat16))
        in_maps.append({"trig": trig_np, "wt": wt_np, "u": u_dev})
    return in_maps


def _out_index():
    # device col for (plane pl, g): h=g>>10, b9=(g>>9)&1, b8=(g>>8)&1, r=g&255
    # col = 2048*h + 1024*b8 + 512*b9 + 256*pl + r;  final[b,a,pl] uses g=GRAY[a]
    g = GRAY
    col = (2048 * (g >> 10) + 1024 * ((g >> 8) & 1) + 512 * ((g >> 9) & 1)
           + (g & 255))
    return np.stack([col, col + 256], axis=-1)     # (2048, 2)


OUT_IDX = _out_index()


def postprocess(results):
    flat = np.concatenate([results[c]["out"] for c in range(NCORES)],
                          axis=0).astype(np.float32)
    return flat[:, OUT_IDX]


def kernel(X, params):
    if "nc" not in _CACHE:
        _CACHE["nc"] = build_kernel()
    nc = _CACHE["nc"]
    in_maps = make_inputs(X, params)
    res = run_bass_kernel_spmd(nc, in_maps, list(range(NCORES)))
    return postprocess(res.results)
